# revision 5
# baseline (speedup 1.0000x reference)
"""2-layer GraphConv (PyG-style) on 8 TRN2 NeuronCores via Bass/Tile. v3.

Strategy (dst-sharded SPMD, one NEFF, bf16 internals):
  - Nodes sharded 2500/core. agg = A@x is computed on RAW features
    (A@(x@W) == (A@x)@W), so the per-edge gather reads the kernel INPUT
    (x_full, bf16 rows) for layer 1 and the AllGather output (h_full)
    for layer 2 -- no y=xW round trip through DRAM at all.
  - Aggregation: edges grouped per (dst-half of 64) into 128-slot chunks;
    gathered rows Xg [128e,128f] are the PE *stationary* operand and a
    one-hot S [128e,64d] (DVE is_equal vs iota) streams as rhs:
      psum[f, wp:wp+64] += Xg.T @ S    (bf16: 64 cy/chunk, fp32 would be 4x)
    into a [128, 512] psum bank (8 halves per bank, 5 banks per layer).
  - Layer finals are small dense matmuls off aggXT (psum->SBUF bf16):
      rows:  h[d,f] = aggXT.T@W_rel + xT.T@W_root + b   (per 128-dst tile)
      flip:  hT[f,d] = W_rel.T@aggXT + W_root.T@xT + b  (per bank, L1 only)
    giving h rows (for the collective) and hT (L2 root lhsT) w/o transposes.
  - ONE AllGather (h, bf16) instead of two fp32 ones, split into `pieces`
    bank-aligned slices so piece p starts as soon as its dst-banks finish;
    node ids are host-remapped into (piece, rank) regions so each collective
    lands contiguously in h_full and layer-2 chunks whose sources live in
    early regions can gather while later pieces are still in flight.
  - L1 chunk order is bank-major (finish banks early -> kick collectives);
    L2 chunk order is piece-major (A-chunks gather under piece b's flight).
    Separate gidx/dval tensors per layer encode the two orders.
"""

import json as _json
import os as _os
import shlex as _shlex


def _apply_cc_workaround():
    """Skip neuronxcc's optional DataLocalityOpt pass: it hits an internal
    assert (NCC_IDLO901) trying to prefetch-localize multi-MB shared gather
    sources. Must run before the jax/axon backend captures compile flags."""
    skip = "--skip-pass=InsertConflictResolutionOps|DataLocalityOpt"

    def fix(flags):
        out = []
        for f in flags:
            if f == skip:
                continue
            if f.startswith("--tensorizer-options=") and skip not in f:
                f = f.rstrip() + " " + skip + " "
            out.append(f)
        return out

    pc_path = _os.environ.get("TRN_TERMINAL_PRECOMPUTED_JSON")
    flags = None
    if pc_path and _os.path.exists(pc_path):
        pc = _json.load(open(pc_path))
        pc["cc_flags"] = fix(pc.get("cc_flags", []))
        _json.dump(pc, open(pc_path, "w"))
        flags = list(pc["cc_flags"])
    try:
        from concourse.compiler_utils import (get_compiler_flags,
                                              set_compiler_flags)
        fl = fix(get_compiler_flags())
        set_compiler_flags(fl)
        if fl:
            _os.environ["NEURON_CC_FLAGS"] = _shlex.join(fl)
    except Exception:
        if flags is not None:
            _os.environ["NEURON_CC_FLAGS"] = _shlex.join(flags)


_apply_cc_workaround()

import ml_dtypes
import numpy as np

import concourse.bacc as bacc
import concourse.mybir as mybir
import concourse.tile as tile
from concourse.bass import AP
from concourse.bass_utils import run_bass_kernel_spmd

F32 = mybir.dt.float32
BF16 = mybir.dt.bfloat16
I16 = mybir.dt.int16
NPBF16 = ml_dtypes.bfloat16

P = 128      # slots per chunk / partitions
W = 64       # dst window width
BANK = 512   # psum bank width (fp32 cols) = dst cols per agg psum tile
D = 128      # feature dim
GPC = 8      # chunks per gather call (8*128 = 1024 idxs)
SGRP = 16    # chunks per S-generation group


def cdiv(a, b):
    return (a + b - 1) // b


# ---------------------------------------------------------------------------
# Host-side preprocessing
# ---------------------------------------------------------------------------

def preprocess(edge_index, n_nodes=20000, n_cores=8, cuts=(0, 1024, 2048, 2500)):
    """Group edges per (dst core, 64-dst half), sort by remapped src, chunk
    into 128-slot pieces; build two instruction orders (L1 bank-major, L2
    piece-major) with per-core gather-index / dst-offset tensors.

    cuts: local-row boundaries of the collective pieces (bank-aligned,
    except the last == npc). Node id remap: local row l of shard c with
    cuts[p] <= l < cuts[p+1] lands at REG[p] + c*(cuts[p+1]-cuts[p]) +
    (l - cuts[p]); regions are ordered by piece.
    """
    npc = n_nodes // n_cores
    n_halves = cdiv(npc, W)
    n_banks = cdiv(npc, BANK)
    n_tiles = cdiv(npc, P)
    pieces = len(cuts) - 1
    assert cuts[0] == 0 and cuts[-1] == npc
    plens = [cuts[p + 1] - cuts[p] for p in range(pieces)]
    regs = np.concatenate([[0], np.cumsum([n_cores * L for L in plens])])

    src = np.asarray(edge_index[0]).astype(np.int64)
    dst = np.asarray(edge_index[1]).astype(np.int64)

    sowner = src // npc
    sloc = src - sowner * npc
    spiece = np.searchsorted(np.asarray(cuts), sloc, side="right") - 1
    spiece = np.clip(spiece, 0, pieces - 1)
    rsrc = (regs[spiece] + sowner * np.asarray(plens)[spiece]
            + (sloc - np.asarray(cuts)[spiece]))

    owner = dst // npc
    dloc = dst - owner * npc
    half = dloc // W

    key = owner * n_halves + half
    order = np.lexsort((rsrc, key))
    key_s, rsrc_s, dloc_s = key[order], rsrc[order], dloc[order]
    bounds = np.searchsorted(key_s, np.arange(n_cores * n_halves + 1))
    counts = (bounds[1:] - bounds[:-1]).reshape(n_cores, n_halves)
    chunks_per_half = cdiv(counts.max(axis=0), P)          # [n_halves]

    # piece class of chunk (h, j): max over cores of the piece of its last
    # real edge (empty/padded -> piece 0)
    n_ch_h = chunks_per_half
    pclass = {}
    for h in range(n_halves):
        for j in range(int(n_ch_h[h])):
            cls = 0
            for c in range(n_cores):
                b0, b1 = bounds[c * n_halves + h], bounds[c * n_halves + h + 1]
                cnt = b1 - b0
                if cnt <= j * P:
                    continue
                last = min(cnt, (j + 1) * P) - 1
                r = rsrc_s[b0 + last]
                cls = max(cls, int(np.searchsorted(regs[1:], r, side="right")))
            pclass[(h, j)] = cls

    def bank_of(h):
        return (h * W) // BANK

    all_chunks = [(h, j) for h in range(n_halves)
                  for j in range(int(n_ch_h[h]))]
    order_l1 = sorted(all_chunks, key=lambda hj: (bank_of(hj[0]), hj[0], hj[1]))
    order_l2 = sorted(all_chunks,
                      key=lambda hj: (pclass[hj], bank_of(hj[0]), hj[0], hj[1]))

    n_real = len(all_chunks)
    nch = cdiv(n_real, GPC) * GPC          # pad to whole gather calls
    n_dummy = nch - n_real
    n_slots = nch * P

    def build_side(order_list):
        """chunk meta (bank, wp, class) in order + per-core gidx/dval."""
        chunks = []
        for (h, j) in order_list:
            chunks.append({"h": h, "j": j, "bank": bank_of(h),
                           "wp": (h * W) % BANK, "cls": pclass[(h, j)],
                           "dummy": False})
        for _ in range(n_dummy):
            chunks.append({"h": -1, "j": -1, "bank": n_banks - 1, "wp": 0,
                           "cls": pieces - 1, "dummy": True})
        per_core = []
        for c in range(n_cores):
            gidx = np.zeros(n_slots, dtype=np.int16)
            dval = np.full(n_slots, -1.0, dtype=np.float32)
            for k, ch in enumerate(chunks):
                if ch["dummy"]:
                    continue
                h, j = ch["h"], ch["j"]
                b0, b1 = bounds[c * n_halves + h], bounds[c * n_halves + h + 1]
                cnt = b1 - b0
                lo, hi = j * P, min(cnt, (j + 1) * P)
                if hi <= lo:
                    continue
                n = hi - lo
                gidx[k * P:k * P + n] = rsrc_s[b0 + lo:b0 + hi]
                dval[k * P:k * P + n] = (dloc_s[b0 + lo:b0 + hi]
                                         - (h * W)).astype(np.float32)
            g16 = gidx.reshape(-1, 16).T                   # [16, n_slots//16]
            gidx_l = np.tile(g16, (8, 1)).astype(np.int16)
            dv_l = dval.reshape(nch, P).T.astype(NPBF16)   # [P, nch]
            per_core.append({"gidx": np.ascontiguousarray(gidx_l),
                             "dval": np.ascontiguousarray(dv_l)})
        # gather-call class: max class among the call's chunks
        call_cls = [max(chunks[k]["cls"]
                        for k in range(m * GPC, (m + 1) * GPC))
                    for m in range(nch // GPC)]
        return chunks, per_core, call_cls

    l1_chunks, l1_pc, _ = build_side(order_l1)
    l2_chunks, l2_pc, l2_call_cls = build_side(order_l2)

    meta = {
        "npc": npc, "n_nodes": n_nodes, "n_cores": n_cores,
        "n_halves": n_halves, "n_banks": n_banks, "n_tiles": n_tiles,
        "pieces": pieces, "cuts": list(cuts), "plens": plens,
        "regs": [int(r) for r in regs],
        "nch": nch, "n_slots": n_slots, "n_real": n_real,
        "l1_chunks": l1_chunks, "l2_chunks": l2_chunks,
        "l2_call_cls": l2_call_cls,
        # remap permutation: x_remap[rpos] = x[node]  (rpos = remap(node))
        "perm": None,
    }
    # node -> remapped position (for x_full row permutation)
    node = np.arange(n_nodes)
    no = node // npc
    nl = node - no * npc
    npiece = np.searchsorted(np.asarray(cuts), nl, side="right") - 1
    npiece = np.clip(npiece, 0, pieces - 1)
    rpos = (regs[npiece] + no * np.asarray(plens)[npiece]
            + (nl - np.asarray(cuts)[npiece]))
    meta["perm"] = rpos.astype(np.int64)                  # len n_nodes
    return meta, l1_pc, l2_pc


# ---------------------------------------------------------------------------
# Kernel builder
# ---------------------------------------------------------------------------

def _bcast3(ap2d, c1n1, c2n2):
    (c1, n1), (c2, n2) = c1n1, c2n2
    return AP(ap2d.tensor, ap2d.offset, [ap2d.ap[0], [c1, n1], [c2, n2]])


def build_kernel(meta, collectives=True, xg_bufs=8, s_bufs=4):
    npc = meta["npc"]
    n_nodes = meta["n_nodes"]
    n_cores = meta["n_cores"]
    n_banks = meta["n_banks"]
    n_tiles = meta["n_tiles"]
    nch = meta["nch"]
    n_slots = meta["n_slots"]
    pieces = meta["pieces"]
    cuts = meta["cuts"]
    plens = meta["plens"]
    regs = meta["regs"]
    npc_pad = n_tiles * P

    nc = bacc.Bacc("TRN2", target_bir_lowering=False, debug=False,
                   num_devices=n_cores)

    # --- I/O ---
    x_full = nc.dram_tensor("x_full", [n_nodes, D], BF16, kind="ExternalInput")
    xT = nc.dram_tensor("xT", [D, npc_pad], BF16, kind="ExternalInput")
    w1r = nc.dram_tensor("w1r", [D, D], BF16, kind="ExternalInput")
    w1o = nc.dram_tensor("w1o", [D, D], BF16, kind="ExternalInput")
    w2r = nc.dram_tensor("w2r", [D, D], BF16, kind="ExternalInput")
    w2o = nc.dram_tensor("w2o", [D, D], BF16, kind="ExternalInput")
    b1 = nc.dram_tensor("b1", [1, D], BF16, kind="ExternalInput")
    b2 = nc.dram_tensor("b2", [1, D], BF16, kind="ExternalInput")
    gidx1 = nc.dram_tensor("gidx1", [P, n_slots // 16], I16,
                           kind="ExternalInput")
    gidx2 = nc.dram_tensor("gidx2", [P, n_slots // 16], I16,
                           kind="ExternalInput")
    dv1 = nc.dram_tensor("dv1", [P, nch], BF16, kind="ExternalInput")
    dv2 = nc.dram_tensor("dv2", [P, nch], BF16, kind="ExternalInput")
    out = nc.dram_tensor("out", [npc, D], F32, kind="ExternalOutput")

    rg = [list(range(n_cores))]
    Relu = mybir.ActivationFunctionType.Relu
    Copy = mybir.ActivationFunctionType.Copy

    with tile.TileContext(nc) as tc:
        with (
            tc.tile_pool(name="const", bufs=1) as constp,
            tc.tile_pool(name="xg", bufs=xg_bufs) as xgp,
            tc.tile_pool(name="sp", bufs=s_bufs) as sp,
            tc.tile_pool(name="aggs", bufs=2) as aggsp,
            tc.tile_pool(name="stage", bufs=4) as stagep,
            tc.tile_pool(name="psagg", bufs=5, space="PSUM") as psagg,
            tc.tile_pool(name="psflip", bufs=1, space="PSUM") as psflip,
            tc.tile_pool(name="psrow", bufs=2, space="PSUM") as psrow,
            tc.tile_pool(name="dram", bufs=1, space="DRAM") as dram,
        ):
            # --- constants / persistent SBUF ---
            w1r_sb = constp.tile([D, D], BF16)
            nc.sync.dma_start(w1r_sb[:], w1r[:])
            w1o_sb = constp.tile([D, D], BF16)
            nc.sync.dma_start(w1o_sb[:], w1o[:])
            w2r_sb = constp.tile([D, D], BF16)
            nc.sync.dma_start(w2r_sb[:], w2r[:])
            w2o_sb = constp.tile([D, D], BF16)
            nc.sync.dma_start(w2o_sb[:], w2o[:])
            b1_sb = constp.tile([1, D], BF16)
            nc.sync.dma_start(b1_sb[:], b1[:])
            b2_sb = constp.tile([1, D], BF16)
            nc.sync.dma_start(b2_sb[:], b2[:])
            ones_sb = constp.tile([1, BANK], BF16)
            nc.gpsimd.memset(ones_sb[:], 1.0)
            zrow_sb = constp.tile([1, D], BF16)
            nc.gpsimd.memset(zrow_sb[:], 0.0)
            xT_sb = constp.tile([D, npc_pad], BF16)
            nc.sync.dma_start(xT_sb[:], xT[:])
            hT_sb = constp.tile([D, npc_pad], BF16)
            if npc_pad > npc:  # zero the pad cols once (read by L2 finals)
                nc.gpsimd.memset(hT_sb[:, npc:], 0.0)
            g1_sb = constp.tile([P, n_slots // 16], I16)
            nc.sync.dma_start(g1_sb[:], gidx1[:])
            g2_sb = constp.tile([P, n_slots // 16], I16)
            nc.sync.dma_start(g2_sb[:], gidx2[:])
            dv1_sb = constp.tile([P, nch], BF16)
            nc.sync.dma_start(dv1_sb[:], dv1[:])
            dv2_sb = constp.tile([P, nch], BF16)
            nc.sync.dma_start(dv2_sb[:], dv2[:])
            iota_i = constp.tile([P, W], mybir.dt.int32)
            nc.gpsimd.iota(iota_i[:], pattern=[[1, W]], base=0,
                           channel_multiplier=0)
            iota_f = constp.tile([P, W], BF16)
            nc.vector.tensor_copy(iota_f[:], iota_i[:])

            # --- DRAM scratch ---
            h_loc = dram.tile([npc, D], BF16)
            # one Shared tensor per collective piece (the sim requires a
            # single writer per Shared mem), copied into the contiguous
            # h_full gather source as each piece lands
            h_piece = [dram.tile([n_cores * plens[p], D], BF16,
                                 addr_space="Shared", name=f"hpiece{p}")
                       for p in range(pieces)]
            h_full = dram.tile([n_nodes, D], BF16)

            def bank_cols(b):
                return min(BANK, npc - b * BANK)

            def gen_s_groups(chunks, dv_sb):
                """is_equal S tiles for runs of SGRP chunks; returns list of
                (s_tile, col offset) indexed by chunk k."""
                smap = []
                for g0 in range(0, nch, SGRP):
                    cnt = min(SGRP, nch - g0)
                    s_t = sp.tile([P, SGRP * W], BF16, tag="smat", name="smat")
                    s3 = AP(s_t.tensor, s_t.offset,
                            [s_t.ap[0], [W, cnt], [1, W]])
                    i3 = _bcast3(iota_f[:, :], [0, cnt], [1, W])
                    d3 = _bcast3(dv_sb[:, g0:g0 + cnt], [1, cnt], [0, W])
                    nc.vector.tensor_tensor(out=s3, in0=i3, in1=d3,
                                            op=mybir.AluOpType.is_equal)
                    for j in range(cnt):
                        smap.append((s_t, j * W))
                return smap

            def agg_layer(chunks, g_sb, src_dram, src_rows_by_cls,
                          bank_open_cb, bank_close_cb):
                """One aggregation pass. chunks: meta list in emission order.
                src_rows_by_cls[cls] = row bound of the gather in_ap for
                chunks of that class. Callbacks fire when a bank's first
                chunk is about to be emitted / after its last chunk."""
                ps = [psagg.tile([P, BANK], F32, tag="psagg", name=f"psagg{b}")
                      for b in range(n_banks)]
                # zero-openers
                for b in range(n_banks):
                    nc.tensor.matmul(ps[b][:, :], lhsT=zrow_sb[:1, :],
                                     rhs=ones_sb[:1, :], start=True,
                                     stop=False)
                last_k_of_bank = {}
                for k, ch in enumerate(chunks):
                    if not ch["dummy"]:
                        last_k_of_bank[ch["bank"]] = k
                xg = None
                for k, ch in enumerate(chunks):
                    if k % GPC == 0:
                        cls = max(chunks[kk]["cls"]
                                  for kk in range(k, k + GPC))
                        rows = src_rows_by_cls[cls]
                        xg = xgp.tile([P, GPC, D], BF16, tag="xg", name="xgbuf")
                        p0 = k * P
                        nc.gpsimd.dma_gather(
                            xg[:, :, :], src_dram[0:rows, :],
                            g_sb[:, p0 // 16:(p0 + GPC * P) // 16],
                            GPC * P, GPC * P, D)
                    if ch["dummy"]:
                        continue
                    b, wp = ch["bank"], ch["wp"]
                    s_t, soff = ch["smap"]
                    nc.tensor.matmul(ps[b][:, wp:wp + W],
                                     lhsT=xg[:, k % GPC, :],
                                     rhs=s_t[:, soff:soff + W],
                                     start=False, stop=False)
                    if last_k_of_bank.get(b) == k:
                        # closer + handoff
                        nc.tensor.matmul(ps[b][:, :], lhsT=zrow_sb[:1, :],
                                         rhs=ones_sb[:1, :], start=False,
                                         stop=True)
                        agg_sb = aggsp.tile([P, BANK], BF16, tag="aggs", name="aggsb")
                        cols = bank_cols(b)
                        nc.scalar.activation(agg_sb[:, :cols],
                                             ps[b][:, :cols], Copy)
                        bank_close_cb(b, agg_sb)

            # ---------------- layer 1 ----------------
            l1_chunks = meta["l1_chunks"]
            smap1 = gen_s_groups(l1_chunks, dv1_sb)
            for k, ch in enumerate(l1_chunks):
                ch["smap"] = smap1[k]

            piece_done_tiles = [cdiv(cuts[p + 1], P) for p in range(pieces)]
            tiles_written = [0]          # h_loc tiles written so far
            coll_issued = [0]            # collective pieces issued

            def maybe_issue_collectives():
                while (coll_issued[0] < pieces and
                       tiles_written[0] >= piece_done_tiles[coll_issued[0]]):
                    p = coll_issued[0]
                    c0, c1 = cuts[p], cuts[p + 1]
                    r0 = regs[p]
                    ln = plens[p]
                    if collectives:
                        nc.gpsimd.collective_compute(
                            "AllGather", mybir.AluOpType.bypass,
                            replica_groups=rg,
                            ins=[h_loc[c0:c1, :].opt()],
                            outs=[h_piece[p][:, :].opt()])
                        nc.sync.dma_start(h_full[r0:r0 + n_cores * ln, :],
                                          h_piece[p][:, :])
                    else:
                        nc.sync.dma_start(h_full[r0:r0 + ln, :],
                                          h_loc[c0:c1, :])
                    coll_issued[0] += 1

            def l1_close(b, agg_sb):
                cols = bank_cols(b)
                # hT (flip): psum[f, d] over this bank's cols
                pf = psflip.tile([P, BANK], F32, tag="psflip", name="psflip")
                nc.tensor.matmul(pf[:, :cols], lhsT=w1r_sb[:],
                                 rhs=agg_sb[:, :cols], start=True, stop=False)
                nc.tensor.matmul(pf[:, :cols], lhsT=w1o_sb[:],
                                 rhs=xT_sb[:, b * BANK:b * BANK + cols],
                                 start=False, stop=False)
                nc.tensor.matmul(pf[:, :cols], lhsT=b1_sb[:1, :],
                                 rhs=ones_sb[:1, :cols], start=False,
                                 stop=True)
                nc.scalar.activation(hT_sb[:, b * BANK:b * BANK + cols],
                                     pf[:, :cols], Relu)
                # h rows per 128-dst tile of this bank
                t0, t1 = (b * BANK) // P, (b * BANK + cols + P - 1) // P
                for t in range(t0, t1):
                    rows = min(P, npc - t * P)
                    toff = t * P - b * BANK
                    pr = psrow.tile([P, D], F32, tag="psrow", name="psrow")
                    nc.tensor.matmul(pr[:, :],
                                     lhsT=agg_sb[:, toff:toff + P],
                                     rhs=w1r_sb[:], start=True, stop=False)
                    nc.tensor.matmul(pr[:, :],
                                     lhsT=xT_sb[:, t * P:(t + 1) * P],
                                     rhs=w1o_sb[:], start=False, stop=False)
                    nc.tensor.matmul(pr[:, :], lhsT=ones_sb[:1, :P],
                                     rhs=b1_sb[:1, :], start=False, stop=True)
                    hr = stagep.tile([P, D], BF16, tag="hrow", name="hrow")
                    nc.scalar.activation(hr[:rows, :], pr[:rows, :], Relu)
                    nc.sync.dma_start(h_loc[t * P:t * P + rows, :],
                                      hr[:rows, :])
                    tiles_written[0] += 1
                maybe_issue_collectives()

            agg_layer(l1_chunks, g1_sb, x_full,
                      {c: n_nodes for c in range(pieces)},
                      lambda b: None, l1_close)
            assert coll_issued[0] == pieces

            # ---------------- layer 2 ----------------
            l2_chunks = meta["l2_chunks"]
            smap2 = gen_s_groups(l2_chunks, dv2_sb)
            for k, ch in enumerate(l2_chunks):
                ch["smap"] = smap2[k]

            rows_by_cls = {p: regs[p + 1] for p in range(pieces)}
            if not collectives:
                rows_by_cls = {p: n_nodes for p in range(pieces)}

            def l2_close(b, agg_sb):
                cols = bank_cols(b)
                t0, t1 = (b * BANK) // P, (b * BANK + cols + P - 1) // P
                for t in range(t0, t1):
                    rows = min(P, npc - t * P)
                    toff = t * P - b * BANK
                    pr = psrow.tile([P, D], F32, tag="psrow", name="psrow")
                    nc.tensor.matmul(pr[:, :],
                                     lhsT=agg_sb[:, toff:toff + P],
                                     rhs=w2r_sb[:], start=True, stop=False)
                    nc.tensor.matmul(pr[:, :],
                                     lhsT=hT_sb[:, t * P:(t + 1) * P],
                                     rhs=w2o_sb[:], start=False, stop=False)
                    nc.tensor.matmul(pr[:, :], lhsT=ones_sb[:1, :P],
                                     rhs=b2_sb[:1, :], start=False, stop=True)
                    ot = stagep.tile([P, D], F32, tag="orow", name="orow")
                    nc.scalar.activation(ot[:rows, :], pr[:rows, :], Copy)
                    nc.sync.dma_start(out[t * P:t * P + rows, :],
                                      ot[:rows, :])

            agg_layer(l2_chunks, g2_sb, h_full, rows_by_cls,
                      lambda b: None, l2_close)

    nc.compile()
    return nc


# ---------------------------------------------------------------------------
# Full-input wrapper
# ---------------------------------------------------------------------------

def make_in_maps(inputs, meta, l1_pc, l2_pc):
    x = np.asarray(inputs["x"], dtype=np.float32)
    n_nodes, _ = x.shape
    npc = meta["npc"]
    n_cores = meta["n_cores"]
    npc_pad = meta["n_tiles"] * P

    x_remap = np.zeros_like(x)
    x_remap[meta["perm"]] = x                 # row n -> position perm[n]
    x_remap = x_remap.astype(NPBF16)

    w1r = np.asarray(inputs["W1_rel"], np.float32).astype(NPBF16)
    w1o = np.asarray(inputs["W1_root"], np.float32).astype(NPBF16)
    w2r = np.asarray(inputs["W2_rel"], np.float32).astype(NPBF16)
    w2o = np.asarray(inputs["W2_root"], np.float32).astype(NPBF16)
    b1 = np.asarray(inputs["b1_rel"], np.float32).astype(NPBF16).reshape(1, D)
    b2 = np.asarray(inputs["b2_rel"], np.float32).astype(NPBF16).reshape(1, D)

    in_maps = []
    for c in range(n_cores):
        xs = x[c * npc:(c + 1) * npc]
        xs_t = np.zeros((D, npc_pad), np.float32)
        xs_t[:, :npc] = xs.T
        in_maps.append({
            "x_full": x_remap,
            "xT": xs_t.astype(NPBF16),
            "w1r": w1r, "w1o": w1o, "w2r": w2r, "w2o": w2o,
            "b1": b1, "b2": b2,
            "gidx1": l1_pc[c]["gidx"], "gidx2": l2_pc[c]["gidx"],
            "dv1": l1_pc[c]["dval"], "dv2": l2_pc[c]["dval"],
        })
    return in_maps


def run(inputs, n_cores=8, trace=False, cuts=(0, 1024, 2048, 2500)):
    _apply_cc_workaround()
    x = np.asarray(inputs["x"], dtype=np.float32)
    meta, l1_pc, l2_pc = preprocess(inputs["edge_index"], x.shape[0],
                                    n_cores, cuts=cuts)
    nc = build_kernel(meta)
    in_maps = make_in_maps(inputs, meta, l1_pc, l2_pc)
    res = run_bass_kernel_spmd(nc, in_maps, core_ids=list(range(n_cores)),
                               trace=trace)
    outp = np.concatenate([res.results[c]["out"] for c in range(n_cores)],
                          axis=0)
    return outp, res


def kernel(**inputs):
    out, _ = run(inputs, n_cores=8)
    return np.asarray(out, dtype=np.float32)


# revision 6
# speedup vs baseline: 1.1521x; 1.1521x over previous
"""2-layer GraphConv (PyG-style) on 8 TRN2 NeuronCores via Bass/Tile. v3.

Strategy (dst-sharded SPMD, one NEFF, bf16 internals):
  - Nodes sharded 2500/core. agg = A@x is computed on RAW features
    (A@(x@W) == (A@x)@W), so the per-edge gather reads the kernel INPUT
    (x_full, bf16 rows) for layer 1 and the AllGather output (h_full)
    for layer 2 -- no y=xW round trip through DRAM at all.
  - Aggregation: edges grouped per (dst-half of 64) into 128-slot chunks;
    gathered rows Xg [128e,128f] are the PE *stationary* operand and a
    one-hot S [128e,64d] (DVE is_equal vs iota) streams as rhs:
      psum[f, wp:wp+64] += Xg.T @ S    (bf16: 64 cy/chunk, fp32 would be 4x)
    into a [128, 512] psum bank (8 halves per bank, 5 banks per layer).
  - Layer finals are small dense matmuls off aggXT (psum->SBUF bf16):
      rows:  h[d,f] = aggXT.T@W_rel + xT.T@W_root + b   (per 128-dst tile)
      flip:  hT[f,d] = W_rel.T@aggXT + W_root.T@xT + b  (per bank, L1 only)
    giving h rows (for the collective) and hT (L2 root lhsT) w/o transposes.
  - ONE AllGather (h, bf16) instead of two fp32 ones, split into `pieces`
    bank-aligned slices so piece p starts as soon as its dst-banks finish;
    node ids are host-remapped into (piece, rank) regions so each collective
    lands contiguously in h_full and layer-2 chunks whose sources live in
    early regions can gather while later pieces are still in flight.
  - L1 chunk order is bank-major (finish banks early -> kick collectives);
    L2 chunk order is piece-major (A-chunks gather under piece b's flight).
    Separate gidx/dval tensors per layer encode the two orders.
"""

import json as _json
import os as _os
import shlex as _shlex


def _apply_cc_workaround():
    """Skip neuronxcc's optional DataLocalityOpt pass: it hits an internal
    assert (NCC_IDLO901) trying to prefetch-localize multi-MB shared gather
    sources. Must run before the jax/axon backend captures compile flags."""
    skip = "--skip-pass=InsertConflictResolutionOps|DataLocalityOpt"

    def fix(flags):
        out = []
        for f in flags:
            if f == skip:
                continue
            if f.startswith("--tensorizer-options=") and skip not in f:
                f = f.rstrip() + " " + skip + " "
            out.append(f)
        return out

    pc_path = _os.environ.get("TRN_TERMINAL_PRECOMPUTED_JSON")
    flags = None
    if pc_path and _os.path.exists(pc_path):
        pc = _json.load(open(pc_path))
        pc["cc_flags"] = fix(pc.get("cc_flags", []))
        _json.dump(pc, open(pc_path, "w"))
        flags = list(pc["cc_flags"])
    try:
        from concourse.compiler_utils import (get_compiler_flags,
                                              set_compiler_flags)
        fl = fix(get_compiler_flags())
        set_compiler_flags(fl)
        if fl:
            _os.environ["NEURON_CC_FLAGS"] = _shlex.join(fl)
    except Exception:
        if flags is not None:
            _os.environ["NEURON_CC_FLAGS"] = _shlex.join(flags)


_apply_cc_workaround()

import ml_dtypes
import numpy as np

import concourse.bacc as bacc
import concourse.mybir as mybir
import concourse.tile as tile
from concourse.bass import AP
from concourse.bass_utils import run_bass_kernel_spmd

F32 = mybir.dt.float32
BF16 = mybir.dt.bfloat16
I16 = mybir.dt.int16
NPBF16 = ml_dtypes.bfloat16

P = 128      # slots per chunk / partitions
W = 128      # dst window width (one 128-dst tile per scatter S-plane)
BANK = 512   # psum bank width (fp32 cols) = dst cols per agg psum tile
D = 128      # feature dim
GPC = 8      # chunks per gather call (8*128 = 1024 idxs)
SGRP = 16    # chunks per S-generation group


def cdiv(a, b):
    return (a + b - 1) // b


# ---------------------------------------------------------------------------
# Host-side preprocessing
# ---------------------------------------------------------------------------

def preprocess(edge_index, n_nodes=20000, n_cores=8, cuts=(0, 512, 1536, 2048, 2500),
               gpc=GPC):
    """Group edges per (dst core, 64-dst half); chunk into 128-slot pieces.

    Layer 1 gathers from x_full (kernel input, no deps): chunks are whole
    sorted halves, ordered bank-major, indices are global remapped rows.
    Layer 2 gathers from the per-piece AllGather outputs: each chunk's
    sources live in ONE piece (sub-chunked at piece boundaries), indices
    are piece-local rows, and chunks are ordered piece-major so class-p
    gathers only depend on collective p (the framework tracks DRAM deps
    per tensor, so distinct piece tensors are what make overlap real).

    cuts: local-row boundaries of the collective pieces (multiples of 128;
    last == npc). Remap: local row l of shard c with cuts[p] <= l <
    cuts[p+1] lands at REG[p] + c*(cuts[p+1]-cuts[p]) + (l - cuts[p]).
    """
    npc = n_nodes // n_cores
    n_halves = cdiv(npc, W)
    n_banks = cdiv(npc, BANK)
    n_tiles = cdiv(npc, P)
    pieces = len(cuts) - 1
    assert cuts[0] == 0 and cuts[-1] == npc
    assert all(c % P == 0 for c in cuts[:-1])
    plens = [cuts[p + 1] - cuts[p] for p in range(pieces)]
    regs = np.concatenate([[0], np.cumsum([n_cores * L for L in plens])])

    src = np.asarray(edge_index[0]).astype(np.int64)
    dst = np.asarray(edge_index[1]).astype(np.int64)

    sowner = src // npc
    sloc = src - sowner * npc
    spiece = np.searchsorted(np.asarray(cuts), sloc, side="right") - 1
    spiece = np.clip(spiece, 0, pieces - 1)
    rsrc = (regs[spiece] + sowner * np.asarray(plens)[spiece]
            + (sloc - np.asarray(cuts)[spiece]))

    owner = dst // npc
    dloc = dst - owner * npc
    half = dloc // W

    key = owner * n_halves + half
    order = np.lexsort((rsrc, key))
    key_s, rsrc_s, dloc_s = key[order], rsrc[order], dloc[order]
    bounds = np.searchsorted(key_s, np.arange(n_cores * n_halves + 1))
    # per (core, half, piece) sub-segment bounds (edges sorted by rsrc,
    # and rsrc regions are piece-ordered)
    pb = np.empty((n_cores, n_halves, pieces + 1), dtype=np.int64)
    for c in range(n_cores):
        for h in range(n_halves):
            b0, b1 = bounds[c * n_halves + h], bounds[c * n_halves + h + 1]
            pb[c, h, 0] = b0
            for p in range(pieces):
                pb[c, h, p + 1] = b0 + np.searchsorted(
                    rsrc_s[b0:b1], regs[p + 1], side="left")
            assert pb[c, h, pieces] == b1

    def bank_of(h):
        return (h * W) // BANK

    def bank_of(h):
        return (h * W) // BANK

    def pack_frames(groups, pad_to):
        """groups: list of dicts {h, bank, wp, cls, cnt (slots), seg_of(c),
        base}. Packs them back-to-back into 128-slot frames (sub-chunks
        never cross frame boundaries). Returns (frames, total_slots) where
        frames[f] = {cls, subs: [(group_idx, slot_in_group, p0, r)]}.
        Total slots padded to pad_to multiple (dead tail frames)."""
        frames = []
        pos = 0                       # global slot cursor
        for gi, g in enumerate(groups):
            # PE weight loads from a non-zero base partition need the
            # array-tiling mode (runtime faults without it), so sub-chunks
            # always start at partition 0 with K=128: pad groups to frames.
            cnt = cdiv(g["cnt"], P) * P
            done = 0
            while done < cnt:
                f, p0 = divmod(pos, P)
                while len(frames) <= f:
                    frames.append({"cls": g["cls"], "subs": []})
                r = min(P - p0, cnt - done)
                frames[f]["cls"] = g["cls"]
                frames[f]["subs"].append((gi, done, p0, r))
                pos += r
                done += r
        n_frames = cdiv(max(pos, 1), P)
        n_frames = cdiv(n_frames, pad_to) * pad_to
        while len(frames) < n_frames:
            frames.append({"cls": groups[-1]["cls"] if groups else 0,
                           "subs": []})
        return frames, n_frames

    def fill_side(groups, frames, n_frames):
        """Per-core gidx/dval tensors for a packed side."""
        n_slots = n_frames * P
        per_core = []
        for c in range(n_cores):
            gidx = np.zeros(n_slots, dtype=np.int16)
            dval = np.full(n_slots, -1.0, dtype=np.float32)
            for f, fr in enumerate(frames):
                for (gi, off, p0, r) in fr["subs"]:
                    g = groups[gi]
                    lo, hi = g["seg"](c)
                    lo = lo + off
                    n = max(0, min(hi, lo + r) - lo)
                    if n <= 0:
                        continue
                    s0 = f * P + p0
                    gidx[s0:s0 + n] = rsrc_s[lo:lo + n] - g["base"]
                    dval[s0:s0 + n] = (dloc_s[lo:lo + n]
                                       - g["h"] * W).astype(np.float32)
            g16 = gidx.reshape(-1, 16).T
            per_core.append({
                "gidx": np.ascontiguousarray(np.tile(g16, (8, 1)).astype(np.int16)),
                "dval": np.ascontiguousarray(
                    dval.reshape(n_frames, P).T.astype(NPBF16))})
        return per_core

    counts = (bounds[1:] - bounds[:-1]).reshape(n_cores, n_halves)

    # ---- layer 1: groups = whole halves (global idx), bank-major ----
    l1_groups = []
    for h in sorted(range(n_halves), key=lambda h: (bank_of(h), h)):
        b0s = bounds[np.arange(n_cores) * n_halves + h]

        def mkseg(h):
            return lambda c: (bounds[c * n_halves + h],
                              bounds[c * n_halves + h + 1])

        l1_groups.append({"h": h, "bank": bank_of(h), "wp": (h * W) % BANK,
                          "cls": 0, "cnt": int(counts[:, h].max()),
                          "seg": mkseg(h), "base": 0})
    l1_frames, nch1 = pack_frames(l1_groups, gpc)
    l1_pc = fill_side(l1_groups, l1_frames, nch1)

    # ---- layer 2: groups = (piece, half) (piece-local idx), piece-major;
    # each phase padded to whole gather calls ----
    l2_groups = []
    l2_frames = []
    nch2 = 0
    phase_nch = []
    for p in range(pieces):
        groups_p = []
        for h in sorted(range(n_halves), key=lambda h: (bank_of(h), h)):
            cnt = int((pb[:, h, p + 1] - pb[:, h, p]).max())
            if cnt == 0:
                continue

            def mkseg(h, p):
                return lambda c: (int(pb[c, h, p]), int(pb[c, h, p + 1]))

            groups_p.append({"h": h, "bank": bank_of(h),
                             "wp": (h * W) % BANK, "cls": p, "cnt": cnt,
                             "seg": mkseg(h, p), "base": int(regs[p])})
        frames_p, n_p = pack_frames(groups_p, gpc)
        for fr in frames_p:
            fr["subs"] = [(gi + len(l2_groups), off, p0, r)
                          for (gi, off, p0, r) in fr["subs"]]
            fr["cls"] = p
        l2_groups.extend(groups_p)
        l2_frames.extend(frames_p)
        phase_nch.append(n_p)
        nch2 += n_p
    l2_pc = fill_side(l2_groups, l2_frames, nch2)

    meta = {
        "npc": npc, "n_nodes": n_nodes, "n_cores": n_cores,
        "n_halves": n_halves, "n_banks": n_banks, "n_tiles": n_tiles,
        "pieces": pieces, "cuts": list(cuts), "plens": plens,
        "regs": [int(r) for r in regs], "gpc": gpc,
        "nch1": nch1, "nch2": nch2, "phase_nch": phase_nch,
        "l1_groups": l1_groups, "l1_frames": l1_frames,
        "l2_groups": l2_groups, "l2_frames": l2_frames,
    }
    node = np.arange(n_nodes)
    no = node // npc
    nl = node - no * npc
    npiece = np.searchsorted(np.asarray(cuts), nl, side="right") - 1
    npiece = np.clip(npiece, 0, pieces - 1)
    rpos = (regs[npiece] + no * np.asarray(plens)[npiece]
            + (nl - np.asarray(cuts)[npiece]))
    meta["perm"] = rpos.astype(np.int64)
    return meta, l1_pc, l2_pc


# ---------------------------------------------------------------------------
# Kernel builder
# ---------------------------------------------------------------------------

def _bcast3(ap2d, c1n1, c2n2):
    (c1, n1), (c2, n2) = c1n1, c2n2
    return AP(ap2d.tensor, ap2d.offset, [ap2d.ap[0], [c1, n1], [c2, n2]])


def build_kernel(meta, collectives=True, xg_bufs=8, s_bufs=4):
    npc = meta["npc"]
    n_nodes = meta["n_nodes"]
    n_cores = meta["n_cores"]
    n_banks = meta["n_banks"]
    n_tiles = meta["n_tiles"]
    nch1, nch2 = meta["nch1"], meta["nch2"]
    pieces = meta["pieces"]
    cuts = meta["cuts"]
    plens = meta["plens"]
    regs = meta["regs"]
    npc_pad = n_tiles * P

    gpc = meta.get("gpc", GPC)
    nc = bacc.Bacc("TRN2", target_bir_lowering=False, debug=False,
                   num_devices=n_cores,
                   dynamic_dma_scratch_size=max(16384, gpc * P * 16))

    # --- I/O ---
    x_full = nc.dram_tensor("x_full", [n_nodes, D], BF16, kind="ExternalInput")
    xT = nc.dram_tensor("xT", [D, npc_pad], BF16, kind="ExternalInput")
    w1r = nc.dram_tensor("w1r", [D, D], BF16, kind="ExternalInput")
    w1o = nc.dram_tensor("w1o", [D, D], BF16, kind="ExternalInput")
    w2r = nc.dram_tensor("w2r", [D, D], BF16, kind="ExternalInput")
    w2o = nc.dram_tensor("w2o", [D, D], BF16, kind="ExternalInput")
    b1 = nc.dram_tensor("b1", [1, D], BF16, kind="ExternalInput")
    b2 = nc.dram_tensor("b2", [1, D], BF16, kind="ExternalInput")
    gidx1 = nc.dram_tensor("gidx1", [P, nch1 * P // 16], I16,
                           kind="ExternalInput")
    gidx2 = nc.dram_tensor("gidx2", [P, nch2 * P // 16], I16,
                           kind="ExternalInput")
    dv1 = nc.dram_tensor("dv1", [P, nch1], BF16, kind="ExternalInput")
    dv2 = nc.dram_tensor("dv2", [P, nch2], BF16, kind="ExternalInput")
    out = nc.dram_tensor("out", [npc, D], F32, kind="ExternalOutput")

    rg = [list(range(n_cores))]
    Relu = mybir.ActivationFunctionType.Relu
    Copy = mybir.ActivationFunctionType.Copy

    with tile.TileContext(nc) as tc:
        with (
            tc.tile_pool(name="const", bufs=1) as constp,
            tc.tile_pool(name="xg", bufs=xg_bufs) as xgp,
            tc.tile_pool(name="sp", bufs=s_bufs) as sp,
            tc.tile_pool(name="aggs", bufs=2) as aggsp,
            tc.tile_pool(name="stage", bufs=4) as stagep,
            tc.tile_pool(name="psagg", bufs=5, space="PSUM") as psagg,
            tc.tile_pool(name="psflip", bufs=1, space="PSUM") as psflip,
            tc.tile_pool(name="psrow", bufs=2, space="PSUM") as psrow,
            tc.tile_pool(name="dram", bufs=1, space="DRAM") as dram,
        ):
            # --- constants / persistent SBUF ---
            w1r_sb = constp.tile([D, D], BF16)
            nc.sync.dma_start(w1r_sb[:], w1r[:])
            w1o_sb = constp.tile([D, D], BF16)
            nc.sync.dma_start(w1o_sb[:], w1o[:])
            w2r_sb = constp.tile([D, D], BF16)
            nc.sync.dma_start(w2r_sb[:], w2r[:])
            w2o_sb = constp.tile([D, D], BF16)
            nc.sync.dma_start(w2o_sb[:], w2o[:])
            b1_sb = constp.tile([1, D], BF16)
            nc.sync.dma_start(b1_sb[:], b1[:])
            b2_sb = constp.tile([1, D], BF16)
            nc.sync.dma_start(b2_sb[:], b2[:])
            ones_sb = constp.tile([1, BANK], BF16)
            nc.gpsimd.memset(ones_sb[:], 1.0)
            zrow_sb = constp.tile([1, D], BF16)
            nc.gpsimd.memset(zrow_sb[:], 0.0)
            xT_sb = constp.tile([D, npc_pad], BF16)
            nc.sync.dma_start(xT_sb[:], xT[:])
            hT_sb = constp.tile([D, npc_pad], BF16)
            if npc_pad > npc:  # zero the pad cols once (read by L2 finals)
                nc.gpsimd.memset(hT_sb[:, npc:], 0.0)
            g1_sb = constp.tile([P, nch1 * P // 16], I16)
            nc.sync.dma_start(g1_sb[:], gidx1[:])
            g2_sb = constp.tile([P, nch2 * P // 16], I16)
            nc.sync.dma_start(g2_sb[:], gidx2[:])
            dv1_sb = constp.tile([P, nch1], BF16)
            nc.sync.dma_start(dv1_sb[:], dv1[:])
            dv2_sb = constp.tile([P, nch2], BF16)
            nc.sync.dma_start(dv2_sb[:], dv2[:])
            # W-major iota plane: value w at position w*SGRP + k (so every
            # is_equal operand keeps a stride-1 last dim -> DVE 2x/4x modes)
            iota_i = constp.tile([P, W * SGRP], mybir.dt.int32)
            i3w = AP(iota_i.tensor, iota_i.offset,
                     [iota_i.ap[0], [SGRP, W], [1, SGRP]])
            nc.gpsimd.iota(i3w, pattern=[[1, W], [0, SGRP]], base=0,
                           channel_multiplier=0)
            iota_f = constp.tile([P, W * SGRP], BF16)
            nc.vector.tensor_copy(iota_f[:], iota_i[:])

            # --- DRAM scratch ---
            # separate tensors per piece: the framework tracks DRAM deps
            # per TENSOR, so piece-p gathers wait only on collective p and
            # collective p waits only on its own h rows
            h_loc = [dram.tile([plens[p], D], BF16, name=f"hloc{p}")
                     for p in range(pieces)]
            h_piece = [dram.tile([n_cores * plens[p], D], BF16,
                                 addr_space="Shared", name=f"hpiece{p}")
                       for p in range(pieces)]

            def bank_cols(b):
                return min(BANK, npc - b * BANK)

            def gen_s_groups(nch, dv_sb):
                """is_equal S tiles for runs of SGRP frames, stored W-major
                (position w*cnt + j): every operand has a stride-1 last dim
                so the DVE runs in its 2x/4x perf mode. Returns per-frame
                matmul rhs APs ([128, W] with column stride cnt)."""
                smap = []
                for g0 in range(0, nch, SGRP):
                    cnt = min(SGRP, nch - g0)
                    s_t = sp.tile([P, W * cnt], BF16, tag="smat", name="smat")
                    s3 = AP(s_t.tensor, s_t.offset,
                            [s_t.ap[0], [cnt, W], [1, cnt]])
                    i3 = AP(iota_f.tensor, iota_f.offset,
                            [iota_f.ap[0], [SGRP, W], [1, cnt]])
                    d3 = _bcast3(dv_sb[:, g0:g0 + cnt], [0, W], [1, cnt])
                    nc.vector.tensor_tensor(out=s3, in0=i3, in1=d3,
                                            op=mybir.AluOpType.is_equal)
                    for j in range(cnt):
                        smap.append(s_t[:, j::cnt])
                return smap

            def agg_layer(groups, frames, smap, g_sb, src_by_cls,
                          bank_close_cb):
                """One aggregation pass over packed frames; each frame has
                one gathered Xg slice and one S plane; its sub-chunks are
                partition-subrange matmuls into their bank windows."""
                ps = [psagg.tile([P, BANK], F32, tag="psagg", name=f"psagg{b}")
                      for b in range(n_banks)]
                for b in range(n_banks):
                    nc.tensor.matmul(ps[b][:, :], lhsT=zrow_sb[:1, :],
                                     rhs=ones_sb[:1, :], start=True,
                                     stop=False)
                last_of_bank = {}
                for f, fr in enumerate(frames):
                    for si, (gi, off, p0, r) in enumerate(fr["subs"]):
                        last_of_bank[groups[gi]["bank"]] = (f, si)
                xg = None
                for f, fr in enumerate(frames):
                    if f % gpc == 0:
                        cls = fr["cls"]
                        src_dram, rows = src_by_cls[cls]
                        xg = xgp.tile([P, gpc, D], BF16, tag="xg", name="xgbuf")
                        s0 = f * P
                        nc.gpsimd.dma_gather(
                            xg[:, :, :], src_dram[0:rows, :],
                            g_sb[:, s0 // 16:(s0 + gpc * P) // 16],
                            gpc * P, gpc * P, D)
                    s_f = smap[f]
                    for si, (gi, off, p0, r) in enumerate(fr["subs"]):
                        g = groups[gi]
                        b, wp = g["bank"], g["wp"]
                        nc.tensor.matmul(ps[b][:, wp:wp + W],
                                         lhsT=xg[p0:p0 + r, f % gpc, :],
                                         rhs=s_f[p0:p0 + r, :],
                                         start=False, stop=False)
                        if last_of_bank.get(b) == (f, si):
                            nc.tensor.matmul(ps[b][:, :], lhsT=zrow_sb[:1, :],
                                             rhs=ones_sb[:1, :], start=False,
                                             stop=True)
                            agg_sb = aggsp.tile([P, BANK], BF16, tag="aggs",
                                                name="aggsb")
                            cols = bank_cols(b)
                            nc.scalar.activation(agg_sb[:, :cols],
                                                 ps[b][:, :cols], Copy)
                            bank_close_cb(b, agg_sb)

            # ---------------- layer 1 ----------------
            smap1 = gen_s_groups(nch1, dv1_sb)

            piece_done_tiles = [cdiv(cuts[p + 1], P) for p in range(pieces)]
            tiles_written = [0]          # h rows tiles written so far
            coll_issued = [0]            # collective pieces issued

            def maybe_issue_collectives():
                while (coll_issued[0] < pieces and
                       tiles_written[0] >= piece_done_tiles[coll_issued[0]]):
                    p = coll_issued[0]
                    if collectives:
                        nc.gpsimd.collective_compute(
                            "AllGather", mybir.AluOpType.bypass,
                            replica_groups=rg,
                            ins=[h_loc[p][:, :].opt()],
                            outs=[h_piece[p][:, :].opt()])
                    else:
                        nc.sync.dma_start(h_piece[p][0:plens[p], :],
                                          h_loc[p][:, :])
                    coll_issued[0] += 1

            def l1_close(b, agg_sb):
                cols = bank_cols(b)
                # hT (flip): psum[f, d] over this bank's cols
                pf = psflip.tile([P, BANK], F32, tag="psflip", name="psflip")
                nc.tensor.matmul(pf[:, :cols], lhsT=w1r_sb[:],
                                 rhs=agg_sb[:, :cols], start=True, stop=False)
                nc.tensor.matmul(pf[:, :cols], lhsT=w1o_sb[:],
                                 rhs=xT_sb[:, b * BANK:b * BANK + cols],
                                 start=False, stop=False)
                nc.tensor.matmul(pf[:, :cols], lhsT=b1_sb[:1, :],
                                 rhs=ones_sb[:1, :cols], start=False,
                                 stop=True)
                nc.scalar.activation(hT_sb[:, b * BANK:b * BANK + cols],
                                     pf[:, :cols], Relu)
                # h rows per 128-dst tile of this bank
                t0, t1 = (b * BANK) // P, (b * BANK + cols + P - 1) // P
                for t in range(t0, t1):
                    rows = min(P, npc - t * P)
                    toff = t * P - b * BANK
                    pr = psrow.tile([P, D], F32, tag="psrow", name="psrow")
                    nc.tensor.matmul(pr[:, :],
                                     lhsT=agg_sb[:, toff:toff + P],
                                     rhs=w1r_sb[:], start=True, stop=False)
                    nc.tensor.matmul(pr[:, :],
                                     lhsT=xT_sb[:, t * P:(t + 1) * P],
                                     rhs=w1o_sb[:], start=False, stop=False)
                    nc.tensor.matmul(pr[:, :], lhsT=ones_sb[:1, :P],
                                     rhs=b1_sb[:1, :], start=False, stop=True)
                    hr = stagep.tile([P, D], BF16, tag="hrow", name="hrow")
                    nc.scalar.activation(hr[:rows, :], pr[:rows, :], Relu)
                    pi = next(p for p in range(pieces)
                              if cuts[p] <= t * P < cuts[p + 1])
                    r0 = t * P - cuts[pi]
                    nc.sync.dma_start(h_loc[pi][r0:r0 + rows, :],
                                      hr[:rows, :])
                    tiles_written[0] += 1
                maybe_issue_collectives()

            agg_layer(meta["l1_groups"], meta["l1_frames"], smap1, g1_sb,
                      {0: (x_full, n_nodes)}, l1_close)
            assert coll_issued[0] == pieces

            # ---------------- layer 2 ----------------
            smap2 = gen_s_groups(nch2, dv2_sb)

            src_by_cls = {p: (h_piece[p], n_cores * plens[p])
                          for p in range(pieces)}

            def l2_close(b, agg_sb):
                cols = bank_cols(b)
                t0, t1 = (b * BANK) // P, (b * BANK + cols + P - 1) // P
                for t in range(t0, t1):
                    rows = min(P, npc - t * P)
                    toff = t * P - b * BANK
                    pr = psrow.tile([P, D], F32, tag="psrow", name="psrow")
                    nc.tensor.matmul(pr[:, :],
                                     lhsT=agg_sb[:, toff:toff + P],
                                     rhs=w2r_sb[:], start=True, stop=False)
                    nc.tensor.matmul(pr[:, :],
                                     lhsT=hT_sb[:, t * P:(t + 1) * P],
                                     rhs=w2o_sb[:], start=False, stop=False)
                    nc.tensor.matmul(pr[:, :], lhsT=ones_sb[:1, :P],
                                     rhs=b2_sb[:1, :], start=False, stop=True)
                    ot = stagep.tile([P, D], F32, tag="orow", name="orow")
                    nc.scalar.activation(ot[:rows, :], pr[:rows, :], Copy)
                    nc.sync.dma_start(out[t * P:t * P + rows, :],
                                      ot[:rows, :])

            agg_layer(meta["l2_groups"], meta["l2_frames"], smap2, g2_sb,
                      src_by_cls, l2_close)

    nc.compile()
    return nc


# ---------------------------------------------------------------------------
# Full-input wrapper
# ---------------------------------------------------------------------------

def make_in_maps(inputs, meta, l1_pc, l2_pc):
    x = np.asarray(inputs["x"], dtype=np.float32)
    n_nodes, _ = x.shape
    npc = meta["npc"]
    n_cores = meta["n_cores"]
    npc_pad = meta["n_tiles"] * P

    x_remap = np.zeros_like(x)
    x_remap[meta["perm"]] = x                 # row n -> position perm[n]
    x_remap = x_remap.astype(NPBF16)

    w1r = np.asarray(inputs["W1_rel"], np.float32).astype(NPBF16)
    w1o = np.asarray(inputs["W1_root"], np.float32).astype(NPBF16)
    w2r = np.asarray(inputs["W2_rel"], np.float32).astype(NPBF16)
    w2o = np.asarray(inputs["W2_root"], np.float32).astype(NPBF16)
    b1 = np.asarray(inputs["b1_rel"], np.float32).astype(NPBF16).reshape(1, D)
    b2 = np.asarray(inputs["b2_rel"], np.float32).astype(NPBF16).reshape(1, D)

    in_maps = []
    for c in range(n_cores):
        xs = x[c * npc:(c + 1) * npc]
        xs_t = np.zeros((D, npc_pad), np.float32)
        xs_t[:, :npc] = xs.T
        in_maps.append({
            "x_full": x_remap,
            "xT": xs_t.astype(NPBF16),
            "w1r": w1r, "w1o": w1o, "w2r": w2r, "w2o": w2o,
            "b1": b1, "b2": b2,
            "gidx1": l1_pc[c]["gidx"], "gidx2": l2_pc[c]["gidx"],
            "dv1": l1_pc[c]["dval"], "dv2": l2_pc[c]["dval"],
        })
    return in_maps


def run(inputs, n_cores=8, trace=False, cuts=(0, 512, 1536, 2048, 2500)):
    _apply_cc_workaround()
    x = np.asarray(inputs["x"], dtype=np.float32)
    meta, l1_pc, l2_pc = preprocess(inputs["edge_index"], x.shape[0],
                                    n_cores, cuts=cuts)
    nc = build_kernel(meta)
    in_maps = make_in_maps(inputs, meta, l1_pc, l2_pc)
    res = run_bass_kernel_spmd(nc, in_maps, core_ids=list(range(n_cores)),
                               trace=trace)
    outp = np.concatenate([res.results[c]["out"] for c in range(n_cores)],
                          axis=0)
    return outp, res


def kernel(**inputs):
    out, _ = run(inputs, n_cores=8)
    return np.asarray(out, dtype=np.float32)
